# revision 12
# baseline (speedup 1.0000x reference)
"""Trainium2 Bass kernel for BrainInspiredAttention.

Math (per batch element b):
    qkv = x_b @ Wqkv.T + bqkv            # [N, 3C]
    q,k,v -> heads [H, N, D]
    scores = (q @ k.T) * neuro[h, m]     # gate broadcast along last axis
    attn  = softmax(scores, -1)
    ctx   = attn @ v                     # [H, N, D] -> [N, C]
    out   = ctx @ Wout.T + bout

Sharding: data-parallel over batch B=8 across the 8 NeuronCores, one batch
element per core.  Host pre-transposes x (-> xT[c,n]) and the weights
(-> WqkvT[c,j], WoutT[c,e]) so every DMA is contiguous and the contraction
dim lands on SBUF partitions without any on-device transposes of big
operands.  All matmuls run as float32r (fp32 storage, PE rounds operands
to 11 mantissa bits, 1 cycle/row -- 4x faster than the fp32 path).

Precision: the PE's fp32r rounding is round-half-up at 11 mantissa bits,
applied to BOTH operands (verified bit-exactly on HW).  Values already
representable in 11 bits pass through exactly, so an error-free hi/lo
compensation works: a = ah + al with ah = round_r(a), al = round_r(a - ah).
The precise variant (default) applies it where softmax amplifies rounding:
the q/k projection (Wh@xh + Wh@xl + Wl@xh) and the score matmuls
(qh.kh + qh.kl + ql.kh), making attention logits ~fp32-exact.  v / attnP /
output-projection matmuls stay single-pass fp32r (their ~2.4e-4 relative
rounding is not amplified).

On-device pipeline per core (J = 3C):
  B: qkT[j,n] = WqkvT.T @ xT for q/k (+ bias, j on partitions), stored as
     hi/lo pair in the precise variant; processed in two n-halves so the
     hi+lo copies of xT fit in SBUF.
  C: v[m,dv]  = xT.T @ WqkvT[:, v-section]        (natural layout, no bias --
     softmax rows sum to 1, so  attn @ (v + bv) == attn @ v + bv  and the v
     bias is added on ctxT instead)
  D: per head h: scores -> gate -> softmax -> attn out; PE-transpose of the
     attn tile; ctxT[d,n] = v.T @ attnT accumulated over m (+ bv)
  E: out[n,e] = ctxT.T @ WoutT + bout
"""

import os
from contextlib import ExitStack

import numpy as np

import concourse.bass as bass
import concourse.tile as tile
from concourse import bacc, mybir
from concourse.bass_utils import run_bass_kernel_spmd
from concourse.masks import make_identity

F32 = mybir.dt.float32
F32R = mybir.dt.float32r
P = 128

# Set by kernel() when KERNEL_TRACE=1; test.py reads it for HW exec time.
LAST_RESULTS = None


def r32(ap):
    """View an f32 AP as float32r for full-rate PE matmuls."""
    return ap.bitcast(F32R)


def round_f32r(a):
    """Round-half-up to 11 mantissa bits -- matches the PE's fp32r operand
    rounding, so the result is read bit-exactly by the hardware."""
    u = np.ascontiguousarray(a, dtype=np.float32).view(np.uint32)
    return ((u + 0x800) & 0xFFFFF000).astype(np.uint32).view(np.float32)


def split_hi_lo(a):
    hi = round_f32r(a)
    lo = round_f32r(a - hi)
    return hi, lo


def build_program(n_cores, N, C, H, precise):
    """Build the SPMD Bass program.  N = seq len, C = model dim, H = heads."""
    D = C // H  # head dim
    CO = C // P        # contraction chunks
    NB = N // P        # seq blocks of 128
    NS = max(N // 512, 1)   # seq slabs of <=512
    SLAB = N // NS     # 512 for full size
    MO = N // P        # m (key) chunks per head
    DB = D // P        # head-dim blocks

    nc = bacc.Bacc("TRN2", target_bir_lowering=False, debug=False,
                   num_devices=n_cores)

    # q/k weight section [C, 2C] (hi/lo pair when precise), v section [C, C]
    Wqkh = nc.dram_tensor("Wqkh", [C, 2 * C], F32R, kind="ExternalInput")
    if precise:
        Wqkl = nc.dram_tensor("Wqkl", [C, 2 * C], F32R, kind="ExternalInput")
        xTl = nc.dram_tensor("xTl", [C, N], F32R, kind="ExternalInput")
    Wv = nc.dram_tensor("Wv", [C, C], F32R, kind="ExternalInput")
    xTh = nc.dram_tensor("xTh", [C, N], F32R, kind="ExternalInput")
    WoutT = nc.dram_tensor("WoutT", [C, C], F32R, kind="ExternalInput")
    bqkv = nc.dram_tensor("bqkv", [3 * C], F32, kind="ExternalInput")
    bout = nc.dram_tensor("bout", [C], F32, kind="ExternalInput")
    neuro = nc.dram_tensor("neuro", [H, N], F32, kind="ExternalInput")

    out = nc.dram_tensor("out", [N, C], F32, kind="ExternalOutput")
    attn = nc.dram_tensor("attn", [H, N, N], F32, kind="ExternalOutput")

    def cview(t):  # [c, j] -> [ci, co, j] with c = co*128 + ci
        return t.ap().rearrange("(co ci) j -> ci co j", ci=P)

    Wqkh_v, Wv_v, Wo_v, xTh_v = cview(Wqkh), cview(Wv), cview(WoutT), cview(xTh)
    if precise:
        Wqkl_v, xTl_v = cview(Wqkl), cview(xTl)
    bqk_v = bqkv.ap()[: 2 * C].rearrange("(jo ji) -> ji jo", ji=P)
    bv_v = bqkv.ap()[2 * C:].rearrange("(vo vi) -> vi vo", vi=P)

    with ExitStack() as top:
        tc = top.enter_context(tile.TileContext(nc))
        dram = top.enter_context(tc.tile_pool(name="dram", bufs=1, space="DRAM"))
        const = top.enter_context(tc.tile_pool(name="const", bufs=1))

        qkTh = dram.tile([2 * C, N], F32R)  # q,k transposed: [j, n], j = h*D+d
        qkTl = dram.tile([2 * C, N], F32R, name="qkTl") if precise else None
        vN = dram.tile([N, C], F32R)        # v natural: [m, h*D + d]
        ctxT = dram.tile([C, N], F32R)      # context transposed: [c', n]

        ident = const.tile([P, P], F32)
        make_identity(nc, ident)
        bqk_t = const.tile([P, 2 * C // P], F32)
        nc.sync.dma_start(out=bqk_t[:], in_=bqk_v)
        bv_t = const.tile([P, C // P], F32)
        nc.sync.dma_start(out=bv_t[:], in_=bv_v)

        # ---- stage B: q/k projection (per n-half when precise) -----------
        _sc, _ = nc.enter_named_scope("proj_qk", False)
        XH = SLAB if precise else N        # n-half width
        JW = 128 if precise else 256       # W tile j-width per DMA
        with tc.tile_pool(name="xth", bufs=1) as xthp, \
             tc.tile_pool(name="xtl", bufs=1) as xtlp, \
             tc.tile_pool(name="wq", bufs=2) as wp, \
             tc.tile_pool(name="proj_sb", bufs=2) as pop, \
             tc.tile_pool(name="proj_ps", bufs=4, space="PSUM") as pp:
            for nh in range(N // XH):
                n0 = nh * XH
                xth = xthp.tile([P, CO, XH], F32R, tag="xth")
                nc.sync.dma_start(out=xth[:], in_=xTh_v[:, :, n0:n0 + XH])
                if precise:
                    xtl = xtlp.tile([P, CO, XH], F32R, tag="xtl")
                    nc.sync.dma_start(out=xtl[:], in_=xTl_v[:, :, n0:n0 + XH])
                for jb in range(0, 2 * C, JW):
                    wh = wp.tile([P, CO, JW], F32R, tag="wh")
                    nc.sync.dma_start(out=wh[:], in_=Wqkh_v[:, :, jb:jb + JW])
                    if precise:
                        wl = wp.tile([P, CO, JW], F32R, tag="wl")
                        nc.sync.dma_start(out=wl[:], in_=Wqkl_v[:, :, jb:jb + JW])
                    for js in range(0, JW, P):
                        for ns in range(XH // SLAB):
                            s0 = ns * SLAB
                            ps = pp.tile([P, SLAB], F32, tag="qk_ps")
                            groups = [(wh, xth)]
                            if precise:
                                groups += [(wh, xtl), (wl, xth)]
                            for gi, (wg, xg) in enumerate(groups):
                                for co in range(CO):
                                    nc.tensor.matmul(
                                        ps[:], wg[:, co, js:js + P],
                                        xg[:, co, s0:s0 + SLAB],
                                        start=(gi == 0 and co == 0),
                                        stop=(gi == len(groups) - 1
                                              and co == CO - 1))
                            jj = (jb + js) // P
                            if precise:
                                t = pop.tile([P, SLAB], F32, tag="t")
                                nc.vector.tensor_scalar(
                                    out=t[:], in0=ps[:],
                                    scalar1=bqk_t[:, jj:jj + 1],
                                    scalar2=None, op0=mybir.AluOpType.add)
                                hi = pop.tile([P, SLAB], F32R, tag="hi")
                                nc.vector.tensor_copy(out=hi[:], in_=t[:])
                                lo = pop.tile([P, SLAB], F32R, tag="lo")
                                nc.vector.tensor_tensor(
                                    lo[:], t[:], hi[:],
                                    mybir.AluOpType.subtract)
                                nc.sync.dma_start(
                                    out=qkTh[jb + js:jb + js + P,
                                             n0 + s0:n0 + s0 + SLAB], in_=hi[:])
                                nc.sync.dma_start(
                                    out=qkTl[jb + js:jb + js + P,
                                             n0 + s0:n0 + s0 + SLAB], in_=lo[:])
                            else:
                                hi = pop.tile([P, SLAB], F32R, tag="hi")
                                nc.vector.tensor_scalar(
                                    out=hi[:], in0=ps[:],
                                    scalar1=bqk_t[:, jj:jj + 1],
                                    scalar2=None, op0=mybir.AluOpType.add)
                                nc.sync.dma_start(
                                    out=qkTh[jb + js:jb + js + P,
                                             n0 + s0:n0 + s0 + SLAB], in_=hi[:])
        nc.leave_named_scope("proj_qk", _sc, False)

        # ---- stage C: v projection (natural layout) ----------------------
        _sc, _ = nc.enter_named_scope("proj_v", False)
        VW = 256
        with tc.tile_pool(name="xtf", bufs=1) as xtfp, \
             tc.tile_pool(name="wv", bufs=2) as wvp, \
             tc.tile_pool(name="v_sb", bufs=2) as vsp, \
             tc.tile_pool(name="v_ps", bufs=4, space="PSUM") as vpp:
            xtf = xtfp.tile([P, CO, N], F32R)
            nc.sync.dma_start(out=xtf[:], in_=xTh_v)
            for db in range(0, C, VW):
                wv = wvp.tile([P, CO, VW], F32R, tag="wv")
                nc.sync.dma_start(out=wv[:], in_=Wv_v[:, :, db:db + VW])
                for mb in range(NB):
                    ps = vpp.tile([P, VW], F32, tag="v_ps")
                    for co in range(CO):
                        nc.tensor.matmul(
                            ps[:], xtf[:, co, mb * P:(mb + 1) * P],
                            wv[:, co, :],
                            start=(co == 0), stop=(co == CO - 1))
                    sb = vsp.tile([P, VW], F32R, tag="v_sb")
                    nc.vector.tensor_copy(out=sb[:], in_=ps[:])
                    nc.sync.dma_start(out=vN[mb * P:(mb + 1) * P, db:db + VW],
                                      in_=sb[:])
        nc.leave_named_scope("proj_v", _sc, False)

        # ---- stage D: attention per head ---------------------------------
        _sc, _ = nc.enter_named_scope("attn", False)
        with tc.tile_pool(name="qk_h", bufs=1) as hp, \
             tc.tile_pool(name="v_h", bufs=1) as vp, \
             tc.tile_pool(name="pt", bufs=1) as ptp, \
             tc.tile_pool(name="soft_sb", bufs=3) as sp, \
             tc.tile_pool(name="stats", bufs=4) as st, \
             tc.tile_pool(name="attn_ps", bufs=3, space="PSUM") as ap_ps, \
             tc.tile_pool(name="tr_ps", bufs=2, space="PSUM") as tr_ps, \
             tc.tile_pool(name="ctx_ps", bufs=3, space="PSUM") as cx_ps, \
             tc.tile_pool(name="ctx_sb", bufs=4) as cxp, \
             tc.tile_pool(name="nr", bufs=1) as nrp:
            for h in range(H):
                def head_load(tag, src):
                    t = hp.tile([P, DB, N], F32R, tag=tag)
                    nc.sync.dma_start(
                        out=t[:],
                        in_=src.rearrange("(do di) n -> di do n", di=P))
                    return t

                qh = head_load("qh", qkTh[h * D:(h + 1) * D, :])
                kh = head_load("kh", qkTh[C + h * D:C + (h + 1) * D, :])
                if precise:
                    ql = head_load("ql", qkTl[h * D:(h + 1) * D, :])
                    kl = head_load("kl", qkTl[C + h * D:C + (h + 1) * D, :])
                vh = vp.tile([P, MO, D], F32R, tag="vh")
                nc.sync.dma_start(
                    out=vh[:], in_=vN[:, h * D:(h + 1) * D]
                    .rearrange("(mo mi) d -> mi mo d", mi=P))
                nr = nrp.tile([P, N], F32, tag="nr")
                nc.sync.dma_start(out=nr[:],
                                  in_=neuro.ap()[h:h + 1, :].to_broadcast((P, N)))

                for ns in range(NS):
                    pt = ptp.tile([P, MO, SLAB], F32R, tag="pt")
                    for nb in range(SLAB // P):
                        n0 = ns * SLAB + nb * P
                        s = sp.tile([P, N], F32, tag="s")
                        # scores[n, m] accumulated over d (hi/lo compensated)
                        for mc in range(NS):
                            ps = ap_ps.tile([P, SLAB], F32, tag="s_ps")
                            pairs = [(qh, kh)]
                            if precise:
                                pairs += [(qh, kl), (ql, kh)]
                            for gi, (qg, kg) in enumerate(pairs):
                                for do in range(DB):
                                    nc.tensor.matmul(
                                        ps[:], qg[:, do, n0:n0 + P],
                                        kg[:, do, mc * SLAB:(mc + 1) * SLAB],
                                        start=(gi == 0 and do == 0),
                                        stop=(gi == len(pairs) - 1
                                              and do == DB - 1))
                            # neurotransmitter gate
                            nc.vector.tensor_mul(
                                out=s[:, mc * SLAB:(mc + 1) * SLAB], in0=ps[:],
                                in1=nr[:, mc * SLAB:(mc + 1) * SLAB])
                        # softmax along m (free axis), in place
                        mx = st.tile([P, 1], F32, tag="mx")
                        nc.vector.tensor_reduce(
                            out=mx[:], in_=s[:], axis=mybir.AxisListType.X,
                            op=mybir.AluOpType.max, negate=True)
                        sm = st.tile([P, 1], F32, tag="sm")
                        nc.scalar.activation(
                            out=s[:], in_=s[:],
                            func=mybir.ActivationFunctionType.Exp,
                            bias=mx[:], scale=1.0, accum_out=sm[:])
                        rs = st.tile([P, 1], F32, tag="rs")
                        nc.vector.reciprocal(out=rs[:], in_=sm[:])
                        nc.vector.tensor_scalar_mul(out=s[:], in0=s[:],
                                                    scalar1=rs[:])
                        nc.sync.dma_start(out=attn[h, n0:n0 + P, :], in_=s[:])
                        # PT[m, n] via PE transposes of 128x128 blocks
                        for mo in range(MO):
                            tp = tr_ps.tile([P, P], F32, tag="tr")
                            nc.tensor.transpose(tp[:], s[:, mo * P:(mo + 1) * P],
                                                ident[:])
                            nc.vector.tensor_copy(out=pt[:, mo, nb * P:(nb + 1) * P],
                                                  in_=tp[:])
                    # ctxT[d, n-slab] = sum_m v[m, d] * PT[m, n]  (+ bv)
                    for dblk in range(DB):
                        cps = cx_ps.tile([P, SLAB], F32, tag="c_ps")
                        for mo in range(MO):
                            nc.tensor.matmul(
                                cps[:], vh[:, mo, dblk * P:(dblk + 1) * P],
                                pt[:, mo, :],
                                start=(mo == 0), stop=(mo == MO - 1))
                        cb = cxp.tile([P, SLAB], F32R, tag="c_sb")
                        vo = (h * D) // P + dblk
                        nc.vector.tensor_scalar(
                            out=cb[:], in0=cps[:], scalar1=bv_t[:, vo:vo + 1],
                            scalar2=None, op0=mybir.AluOpType.add)
                        nc.sync.dma_start(
                            out=ctxT[h * D + dblk * P:h * D + (dblk + 1) * P,
                                     ns * SLAB:(ns + 1) * SLAB],
                            in_=cb[:])
        nc.leave_named_scope("attn", _sc, False)

        # ---- stage E: output projection ----------------------------------
        _sc, _ = nc.enter_named_scope("outproj", False)
        EBLK = 256
        with tc.tile_pool(name="ctf", bufs=1) as ctf, \
             tc.tile_pool(name="wo", bufs=2) as wop, \
             tc.tile_pool(name="ob", bufs=2) as obp, \
             tc.tile_pool(name="out_sb", bufs=4) as osp, \
             tc.tile_pool(name="out_ps", bufs=6, space="PSUM") as ops:
            ct = ctf.tile([P, CO, N], F32R)
            nc.sync.dma_start(out=ct[:],
                              in_=ctxT[:].rearrange("(co ci) n -> ci co n", ci=P))
            for eb in range(0, C, EBLK):
                wo = wop.tile([P, CO, EBLK], F32R, tag="wo")
                nc.sync.dma_start(out=wo[:], in_=Wo_v[:, :, eb:eb + EBLK])
                ob = obp.tile([P, EBLK], F32, tag="ob")
                nc.sync.dma_start(out=ob[:],
                                  in_=bout.ap()[None, eb:eb + EBLK]
                                  .to_broadcast((P, EBLK)))
                for nb in range(NB):
                    ps = ops.tile([P, EBLK], F32, tag="o_ps")
                    for co in range(CO):
                        nc.tensor.matmul(
                            ps[:], ct[:, co, nb * P:(nb + 1) * P],
                            wo[:, co, :],
                            start=(co == 0), stop=(co == CO - 1))
                    osb = osp.tile([P, EBLK], F32, tag="o_sb")
                    nc.vector.tensor_add(out=osb[:], in0=ps[:], in1=ob[:])
                    nc.sync.dma_start(out=out[nb * P:(nb + 1) * P, eb:eb + EBLK],
                                      in_=osb[:])
        nc.leave_named_scope("outproj", _sc, False)

    nc.compile()
    return nc


_PROGRAM_CACHE = {}


def _get_program(n_cores, N, C, H, precise):
    key = (n_cores, N, C, H, precise)
    if key not in _PROGRAM_CACHE:
        _PROGRAM_CACHE[key] = build_program(n_cores, N, C, H, precise)
    return _PROGRAM_CACHE[key]


def kernel(x, Wqkv, bqkv, Wout, bout, neuro):
    """Full-input entry point. x [B,N,C]; returns (out [B,N,C], attn [B,H,N,N])."""
    global LAST_RESULTS
    x = np.ascontiguousarray(np.asarray(x, dtype=np.float32))
    Wqkv = np.asarray(Wqkv, dtype=np.float32)
    bqkv = np.ascontiguousarray(np.asarray(bqkv, dtype=np.float32))
    Wout = np.asarray(Wout, dtype=np.float32)
    bout = np.ascontiguousarray(np.asarray(bout, dtype=np.float32))
    neuro = np.ascontiguousarray(np.asarray(neuro, dtype=np.float32))

    B, N, C = x.shape
    H = neuro.shape[0]
    precise = os.environ.get("KERNEL_FAST", "") != "1"

    nc = _get_program(B, N, C, H, precise)

    WqkvT = np.ascontiguousarray(Wqkv.T)   # [C, 3C]
    WoutT = np.ascontiguousarray(Wout.T)   # [C, C]
    xT = np.ascontiguousarray(x.transpose(0, 2, 1))  # [B, C, N]

    Wqk = np.ascontiguousarray(WqkvT[:, :2 * C])
    Wv = np.ascontiguousarray(WqkvT[:, 2 * C:])
    common = {"Wv": Wv, "WoutT": WoutT, "bqkv": bqkv, "bout": bout,
              "neuro": neuro}
    if precise:
        Wqkh, Wqkl = split_hi_lo(Wqk)
        common.update({"Wqkh": Wqkh, "Wqkl": Wqkl})
        xs = [dict(common) for _ in range(B)]
        for b in range(B):
            xh, xl = split_hi_lo(xT[b])
            xs[b].update({"xTh": xh, "xTl": xl})
        in_maps = xs
    else:
        common["Wqkh"] = Wqk
        in_maps = [{**common, "xTh": xT[b]} for b in range(B)]

    trace = os.environ.get("KERNEL_TRACE", "") == "1"
    res = run_bass_kernel_spmd(nc, in_maps, core_ids=list(range(B)),
                               trace=trace)
    LAST_RESULTS = res

    out = np.stack([res.results[b]["out"] for b in range(B)])
    attn = np.stack([res.results[b]["attn"] for b in range(B)])
    return out, attn
